# revision 16
# baseline (speedup 1.0000x reference)
# Trainium2 Bass kernel for nn_Net_89687507075560 (ResNet feature extractor + top-2 MoE head).
# Data-parallel over batch: 512 images -> 8 cores x 64 images, params replicated.
# Convs run as fp32r matmuls (full-rate, ~12-bit-mantissa fp32) with fp32 PSUM accumulation.
# Stage A (32x32, C<=64) uses 2-group row/col array tiling (imgs split across partition halves).
# Activations spill to DRAM between stages (SBUF working set per stage stays < 200KB/partition).
import sys

sys.path.insert(0, "/opt/trn_rl_repo")

import numpy as np

N_CORES = 8
EPS = 1e-5
ROUND_ALL = False  # add explicit fp32r rounding copies after memset/DMA producers

# ---------------------------------------------------------------------------
# Host-side weight preparation
# ---------------------------------------------------------------------------

def _np(a):
    return np.asarray(a, dtype=np.float32)


def _fold_bn(p):
    g, b, m, v = _np(p["g"]), _np(p["b"]), _np(p["m"]), _np(p["v"])
    inv = g / np.sqrt(v + EPS)
    return inv, b - m * inv


def _conv_w_tile(W, replicate):
    O, I = W.shape[0], W.shape[1]
    out = np.zeros((128, 9 * O), dtype=np.float32)
    for ky in range(3):
        for kx in range(3):
            j = ky * 3 + kx
            wj = W[:, :, ky, kx].T
            out[:I, j * O:(j + 1) * O] = wj
            if replicate:
                out[64:64 + I, j * O:(j + 1) * O] = wj
    return out


def _conv_w_tile_c(W):
    O = W.shape[0]
    out = np.zeros((128, 2 * 9 * O), dtype=np.float32)
    for c in range(2):
        for ky in range(3):
            for kx in range(3):
                j = ky * 3 + kx
                out[:, (c * 9 + j) * O:(c * 9 + j + 1) * O] = W[:, c * 128:(c + 1) * 128, ky, kx].T
    return out


def _sc_w_tile(W, replicate):
    O, I = W.shape[0], W.shape[1]
    out = np.zeros((128, O), dtype=np.float32)
    out[:I, :] = W[:, :, 0, 0].T
    if replicate:
        out[64:64 + I, :] = W[:, :, 0, 0].T
    return out


def _bn_cols(bnv, colmap, key, inv, bias, replicate, nchunk=1):
    C = inv.shape[0]
    if nchunk == 1:
        s = np.zeros(128, np.float32)
        b = np.zeros(128, np.float32)
        s[:C] = inv
        b[:C] = bias
        if replicate:
            s[64:64 + C] = inv
            b[64:64 + C] = bias
        colmap[key] = (len(bnv), 1)
        bnv.append(s)
        bnv.append(b)
    else:
        colmap[key] = (len(bnv), 2)
        bnv.append(inv[:128])
        bnv.append(inv[128:])
        bnv.append(bias[:128])
        bnv.append(bias[128:])


def prep_shared(params):
    arrs = {}
    p = params
    arrs["w27"] = _np(p["conv1"]).transpose(2, 3, 1, 0).reshape(27, 16).copy()

    bnv = []
    colmap = {}

    for bi, bp in enumerate(p["block1"]):
        inv1, b1 = _fold_bn(bp["bn1"])
        inv2, b2 = _fold_bn(bp["bn2"])
        _bn_cols(bnv, colmap, f"A{bi}bn1", inv1, b1, replicate=True)
        _bn_cols(bnv, colmap, f"A{bi}bn2", inv2, b2, replicate=True)
        arrs[f"wA{bi}c1"] = _conv_w_tile(_np(bp["conv1"]), replicate=True)
        arrs[f"wA{bi}c2"] = _conv_w_tile(_np(bp["conv2"]), replicate=True)
        if "shortcut" in bp:
            arrs[f"wA{bi}sc"] = _sc_w_tile(_np(bp["shortcut"]), replicate=True)
    for bi, bp in enumerate(p["block2"]):
        inv1, b1 = _fold_bn(bp["bn1"])
        inv2, b2 = _fold_bn(bp["bn2"])
        _bn_cols(bnv, colmap, f"B{bi}bn1", inv1, b1, replicate=(bi == 0))
        _bn_cols(bnv, colmap, f"B{bi}bn2", inv2, b2, replicate=False)
        arrs[f"wB{bi}c1"] = _conv_w_tile(_np(bp["conv1"]), replicate=(bi == 0))
        arrs[f"wB{bi}c2"] = _conv_w_tile(_np(bp["conv2"]), replicate=False)
        if "shortcut" in bp:
            arrs[f"wB{bi}sc"] = _sc_w_tile(_np(bp["shortcut"]), replicate=(bi == 0))
    for bi, bp in enumerate(p["block3"]):
        inv1, b1 = _fold_bn(bp["bn1"])
        inv2, b2 = _fold_bn(bp["bn2"])
        _bn_cols(bnv, colmap, f"C{bi}bn1", inv1, b1, replicate=False, nchunk=1 if bi == 0 else 2)
        _bn_cols(bnv, colmap, f"C{bi}bn2", inv2, b2, replicate=False, nchunk=2)
        if bi == 0:
            arrs[f"wC{bi}c1"] = _conv_w_tile(_np(bp["conv1"]), replicate=False)
        else:
            arrs[f"wC{bi}c1"] = _conv_w_tile_c(_np(bp["conv1"]))
        arrs[f"wC{bi}c2"] = _conv_w_tile_c(_np(bp["conv2"]))
        if "shortcut" in bp:
            arrs[f"wC{bi}sc"] = _sc_w_tile(_np(bp["shortcut"]), replicate=False)
    invf, bf = _fold_bn(p["bn_final"])
    _bn_cols(bnv, colmap, "bnf", invf / 64.0, bf / 64.0, replicate=False, nchunk=2)

    bn_arr = np.zeros((128, len(bnv)), dtype=np.float32)
    for i, col in enumerate(bnv):
        bn_arr[:, i] = col
    arrs["bnv"] = bn_arr

    arrs["ident"] = np.eye(128, dtype=np.float32)

    gp = p["gate"]
    T = float(np.clip(_np(gp["temperature"])[0], 0.5, 2.0))
    w1, gb1 = _np(gp["w1"]), _np(gp["b1"])
    w2, gb2 = _np(gp["w2"]) / T, _np(gp["b2"]) / T
    ep = p["experts"]
    fc1w = _np(ep["fc1_w"])
    fc1b = _np(ep["fc1_b"])
    inv1e = np.zeros((8, 256), np.float32)
    bia1e = np.zeros((8, 256), np.float32)
    g_, b_, m_, v_ = _np(ep["bn1"]["g"]), _np(ep["bn1"]["b"]), _np(ep["bn1"]["m"]), _np(ep["bn1"]["v"])
    for e in range(8):
        iv = g_[e] / np.sqrt(v_[e] + EPS)
        inv1e[e] = iv
        bia1e[e] = fc1b[e] * iv + (b_[e] - m_[e] * iv)
    fc2w = _np(ep["fc2_w"])
    fc2b = _np(ep["fc2_b"])
    inv2e = np.zeros((8, 128), np.float32)
    bia2e = np.zeros((8, 128), np.float32)
    g_, b_, m_, v_ = _np(ep["bn2"]["g"]), _np(ep["bn2"]["b"]), _np(ep["bn2"]["m"]), _np(ep["bn2"]["v"])
    for e in range(8):
        iv = g_[e] / np.sqrt(v_[e] + EPS)
        inv2e[e] = iv
        bia2e[e] = fc2b[e] * iv + (b_[e] - m_[e] * iv)
    fc3w = _np(ep["fc3_w"])
    fc3b = _np(ep["fc3_b"])

    a = np.zeros((128, 8 * 2 * 2 * 128), np.float32)
    for e in range(8):
        for c in range(2):
            for mt in range(2):
                a[:, ((e * 2 + c) * 2 + mt) * 128:((e * 2 + c) * 2 + mt + 1) * 128] = \
                    fc1w[e, c * 128:(c + 1) * 128, mt * 128:(mt + 1) * 128]
    arrs["fc1w"] = a
    a = np.zeros((128, 8 * 2 * 128), np.float32)
    for e in range(8):
        for hc in range(2):
            a[:, (e * 2 + hc) * 128:(e * 2 + hc + 1) * 128] = fc2w[e, hc * 128:(hc + 1) * 128, :]
    arrs["fc2w"] = a
    emr = np.zeros((128, 354), np.float32)
    for e in range(8):
        emr[:, e * 10:(e + 1) * 10] = fc3w[e]
    emr[:8, 80:90] = fc3b
    for c in range(2):
        emr[:, 90 + c * 128:90 + (c + 1) * 128] = w1[c * 128:(c + 1) * 128, :]
    emr[:, 346:354] = w2
    arrs["emr"] = emr
    emf = np.zeros((128, 50), np.float32)
    for e in range(8):
        for mt in range(2):
            emf[:, e * 2 + mt] = inv1e[e, mt * 128:(mt + 1) * 128]
            emf[:, 16 + e * 2 + mt] = bia1e[e, mt * 128:(mt + 1) * 128]
        emf[:, 32 + e] = inv2e[e]
        emf[:, 40 + e] = bia2e[e]
    emf[:, 48] = gb1
    emf[:8, 49] = gb2
    arrs["emf"] = emf

    return arrs, colmap


def prep_xcol(x_core):
    NI = x_core.shape[0]
    xp = np.zeros((NI, 3, 34, 34), dtype=np.float32)
    xp[:, :, 1:33, 1:33] = x_core
    out = np.empty((27, NI * 1024), dtype=np.float32)
    for ky in range(3):
        for kx in range(3):
            j = ky * 3 + kx
            out[j * 3:(j + 1) * 3, :] = (
                xp[:, :, ky:ky + 32, kx:kx + 32].transpose(1, 0, 2, 3).reshape(3, NI * 1024)
            )
    return out


# ---------------------------------------------------------------------------
# Kernel emission
# ---------------------------------------------------------------------------

OFFS = [(ky, kx) for ky in range(3) for kx in range(3)]


def build_nc(colmap, nimg=64, reps=None):
    import concourse.bacc as bacc
    import concourse.tile as tile
    import concourse.mybir as mybir
    from concourse.alu_op_type import AluOpType as Alu
    import bass_rust
    from contextlib import ExitStack, nullcontext

    f32 = mybir.dt.float32
    f32r = mybir.dt.float32r
    Relu = mybir.ActivationFunctionType.Relu
    Iden = mybir.ActivationFunctionType.Identity
    Exp = mybir.ActivationFunctionType.Exp
    AX = bass_rust.AxisListType

    nc = bacc.Bacc("TRN2", target_bir_lowering=False, debug=False, num_devices=N_CORES)

    shapes = {
        "w27": [27, 16], "ident": [128, 128],
        "fc1w": [128, 4096], "fc2w": [128, 2048], "emr": [128, 354], "emf": [128, 50],
        "xcol": [27, nimg * 1024],
    }
    wshapes = {}
    for bi in range(4):
        wshapes[f"wA{bi}c1"] = [128, 576]
        wshapes[f"wA{bi}c2"] = [128, 576]
    wshapes["wA0sc"] = [128, 64]
    wshapes["wB0sc"] = [128, 128]
    for bi in range(4):
        wshapes[f"wB{bi}c1"] = [128, 1152]
        wshapes[f"wB{bi}c2"] = [128, 1152]
    wshapes["wC0c1"] = [128, 2304]
    wshapes["wC0sc"] = [128, 256]
    wshapes["wC0c2"] = [128, 4608]
    for bi in range(1, 4):
        wshapes[f"wC{bi}c1"] = [128, 4608]
        wshapes[f"wC{bi}c2"] = [128, 4608]

    NB = 2 * sum(v[1] for v in colmap.values())
    shapes["bnv"] = [128, NB]

    d = {}
    rnd_names = set(wshapes) | {"fc1w", "fc2w", "emr"}
    for name, shp in {**shapes, **wshapes}.items():
        dt_ = f32r if name in rnd_names else f32
        d[name] = nc.dram_tensor(name, shp, dt_, kind="ExternalInput")
    d["out"] = nc.dram_tensor("out", [nimg, 10], f32, kind="ExternalOutput")
    d["dB_cv1"] = nc.dram_tensor("dB_cv1", [128, nimg * 324], f32)
    d["dB_sc"] = nc.dram_tensor("dB_sc", [128, nimg * 256], f32)
    d["dC_cv1"] = nc.dram_tensor("dC_cv1", [128, 2 * nimg * 100], f32)
    d["dC_sc"] = nc.dram_tensor("dC_sc", [128, 2 * nimg * 64], f32)

    ASUB, BSUB, CSUB = 8, 16, 16
    nA, nB_, nC = nimg // ASUB, nimg // BSUB, nimg // CSUB

    with tile.TileContext(nc) as tc, ExitStack() as ctx:
        pact = ctx.enter_context(tc.tile_pool(name="acts", bufs=1))
        psml = ctx.enter_context(tc.tile_pool(name="small", bufs=1))
        pw = ctx.enter_context(tc.tile_pool(name="wst", bufs=3))
        pstg = ctx.enter_context(tc.tile_pool(name="stg", bufs=3))
        pxc = ctx.enter_context(tc.tile_pool(name="xc", bufs=2))
        pps = ctx.enter_context(tc.tile_pool(name="cps", bufs=4, space="PSUM"))
        ppse = ctx.enter_context(tc.tile_pool(name="eps", bufs=2, space="PSUM"))
        ppst = ctx.enter_context(tc.tile_pool(name="tps", bufs=2, space="PSUM"))

        bnv = psml.tile([128, NB], f32, tag="bnv")
        nc.sync.dma_start(bnv[:], d["bnv"][:, :])
        ident = psml.tile([128, 128], f32, tag="ident")
        nc.sync.dma_start(ident[:], d["ident"][:, :])
        emf = psml.tile([128, 50], f32, tag="emf")
        nc.sync.dma_start(emf[:], d["emf"][:, :])
        fT = psml.tile([128, 2 * nimg], f32, tag="fT")
        fTr = psml.tile([128, 2 * nimg], f32r, tag="fTr")
        ZC = max(4 * 1156, BSUB * 324, 2 * CSUB * 100) // 2
        zeros = psml.tile([128, ZC], f32, tag="zeros")
        nc.gpsimd.memset(zeros[:], 0.0)

        def bnS(key, c=0):
            col, nch = colmap[key]
            return bnv[:, col + c:col + c + 1]

        def bnB(key, c=0):
            col, nch = colmap[key]
            return bnv[:, col + nch + c:col + nch + c + 1]

        def load_w(name, rnd=True):
            shp = wshapes.get(name) or shapes[name]
            t = pw.tile([128, shp[1]], f32r if rnd else f32, tag="wst")
            nc.sync.dma_start(t[0:shp[0], :], d[name][:, :])
            return t

        def act_tile(tag, cols):
            t = pact.tile([128, cols], f32r, tag=tag)
            return t

        def zero_r(t):
            # f32r memset is invalid ISA; produce zeros via DVE copy (a "rounding" op)
            half = t.shape[1] // 2
            nc.vector.tensor_copy(t[:, 0:half], zeros[:, 0:half])
            nc.vector.tensor_copy(t[:, half:t.shape[1]], zeros[:, 0:t.shape[1] - half])

        if reps is not None:
            ctx.enter_context(tc.For_i(0, reps, 1))

        # ============ PHASE A ============
        for sb in range(nA):
            bufs = [act_tile(f"act{i}", 4 * 1156) for i in range(4)]
            for b in bufs:
                zero_r(b)
            res, tbuf, hbuf, t2buf = bufs

            def aview(t):
                return t[:].rearrange("p (i y x) -> p i y x", i=4, y=34, x=34)

            rv, tv, hv, t2v = aview(res), aview(tbuf), aview(hbuf), aview(t2buf)

            # conv1 (3->16), plain fp32, both groups via psum col split
            w27 = load_w("w27", rnd=False)
            xv = d["xcol"][:, :].rearrange("p (i n) -> p i n", i=nimg, n=1024)
            for il in range(4):
                xc = pxc.tile([27, 2048], f32, tag="xc")
                g1 = sb * ASUB + il
                g2 = sb * ASUB + 4 + il
                nc.sync.dma_start(xc[:, 0:1024], xv[:, g1, :])
                nc.sync.dma_start(xc[:, 1024:2048], xv[:, g2, :])
                for h in range(2):
                    psA = pps.tile([128, 512], f32, tag="cps")
                    psB = pps.tile([128, 512], f32, tag="cps")
                    nc.tensor.matmul(psA[0:16, :], w27[0:27, :], xc[0:27, h * 512:(h + 1) * 512],
                                     start=True, stop=True)
                    nc.tensor.matmul(psB[0:16, :], w27[0:27, :], xc[0:27, 1024 + h * 512:1536 + h * 512],
                                     start=True, stop=True)
                    pvA = psA[:].rearrange("p (y x) -> p y x", y=16, x=32)
                    pvB = psB[:].rearrange("p (y x) -> p y x", y=16, x=32)
                    nc.scalar.copy(rv[0:16, il, 1 + 16 * h:17 + 16 * h, 1:33], pvA[0:16, :, :])
                    nc.scalar.copy(rv[64:80, il, 1 + 16 * h:17 + 16 * h, 1:33], pvB[0:16, :, :])

            for bi in range(4):
                Ic = 16 if bi == 0 else 64
                wc1 = load_w(f"wA{bi}c1")
                wc2 = load_w(f"wA{bi}c2")
                wsc = load_w("wA0sc") if bi == 0 else None

                for il in range(4):
                    nc.scalar.activation(tv[:, il, 1:33, 1:33], rv[:, il, 1:33, 1:33], Relu,
                                         bias=bnB(f"A{bi}bn1"), scale=bnS(f"A{bi}bn1"))
                for il in range(4):
                    for h in range(2):
                        psA = pps.tile([128, 512], f32, tag="cps")
                        psB = pps.tile([128, 512], f32, tag="cps")
                        for j, (ky, kx) in enumerate(OFFS):
                            for pb, ps in ((0, psA), (64, psB)):
                                nc.tensor.matmul(
                                    ps[0:64, :],
                                    wc1[pb:pb + Ic, j * 64:(j + 1) * 64],
                                    tv[pb:pb + Ic, il, 16 * h + ky:16 * h + ky + 16, kx:kx + 32],
                                    start=(j == 0), stop=(j == 8))
                        pvA = psA[:].rearrange("p (y x) -> p y x", y=16, x=32)
                        pvB = psB[:].rearrange("p (y x) -> p y x", y=16, x=32)
                        nc.scalar.copy(hv[0:64, il, 1 + 16 * h:17 + 16 * h, 1:33], pvA[0:64, :, :])
                        nc.scalar.copy(hv[64:128, il, 1 + 16 * h:17 + 16 * h, 1:33], pvB[0:64, :, :])
                for il in range(4):
                    nc.scalar.activation(t2v[:, il, 1:33, 1:33], hv[:, il, 1:33, 1:33], Relu,
                                         bias=bnB(f"A{bi}bn2"), scale=bnS(f"A{bi}bn2"))
                for il in range(4):
                    for h in range(2):
                        psA = pps.tile([128, 512], f32, tag="cps")
                        psB = pps.tile([128, 512], f32, tag="cps")
                        for j, (ky, kx) in enumerate(OFFS):
                            for pb, ps in ((0, psA), (64, psB)):
                                nc.tensor.matmul(
                                    ps[0:64, :],
                                    wc2[pb:pb + 64, j * 64:(j + 1) * 64],
                                    t2v[pb:pb + 64, il, 16 * h + ky:16 * h + ky + 16, kx:kx + 32],
                                    start=(j == 0), stop=(j == 8 and wsc is None))
                        if wsc is not None:
                            for pb, ps in ((0, psA), (64, psB)):
                                nc.tensor.matmul(
                                    ps[0:64, :],
                                    wsc[pb:pb + Ic, 0:64],
                                    rv[pb:pb + Ic, il, 1 + 16 * h:17 + 16 * h, 1:33],
                                    start=False, stop=True)
                        for pb, ps in ((0, psA), (64, psB)):
                            pv = ps[:].rearrange("p (y x) -> p y x", y=16, x=32)
                            dst = tv[pb:pb + 64, il, 1 + 16 * h:17 + 16 * h, 1:33]
                            if wsc is not None:
                                nc.vector.tensor_copy(dst, pv[0:64, :, :])
                            else:
                                nc.vector.tensor_add(dst, pv[0:64, :, :],
                                                     rv[pb:pb + 64, il, 1 + 16 * h:17 + 16 * h, 1:33])
                res, tbuf = tbuf, res
                rv, tv = tv, rv

            # transition A->B: t = relu(bn1_B0(res)); cv1 = conv_s2(t); sc = conv1x1_s2(res)
            wt = load_w("wB0c1")
            wts = load_w("wB0sc")
            for il in range(4):
                nc.scalar.activation(tv[:, il, 1:33, 1:33], rv[:, il, 1:33, 1:33], Relu,
                                     bias=bnB("B0bn1"), scale=bnS("B0bn1"))
            for grp in range(2):
                pb = grp * 64
                for il2 in range(2):
                    imgbase = sb * ASUB + grp * 4 + il2 * 2
                    ps = pps.tile([128, 512], f32, tag="cps")
                    for j, (ky, kx) in enumerate(OFFS):
                        nc.tensor.matmul(
                            ps[:, :],
                            wt[pb:pb + 64, j * 128:(j + 1) * 128],
                            tv[pb:pb + 64, il2 * 2:il2 * 2 + 2, ky:ky + 32:2, kx:kx + 32:2],
                            start=(j == 0), stop=(j == 8))
                    stg = pstg.tile([128, 648], f32, tag="stg")
                    nc.gpsimd.memset(stg[:], 0.0)
                    sv = stg[:].rearrange("p (i y x) -> p i y x", i=2, y=18, x=18)
                    pv = ps[:].rearrange("p (i y x) -> p i y x", i=2, y=16, x=16)
                    nc.vector.tensor_copy(sv[:, :, 1:17, 1:17], pv[:, :, :, :])
                    nc.sync.dma_start(d["dB_cv1"][:, imgbase * 324:(imgbase + 2) * 324], stg[:])
                    ps2 = pps.tile([128, 512], f32, tag="cps")
                    nc.tensor.matmul(
                        ps2[:, :],
                        wts[pb:pb + 64, 0:128],
                        rv[pb:pb + 64, il2 * 2:il2 * 2 + 2, 1:33:2, 1:33:2],
                        start=True, stop=True)
                    stg2 = pstg.tile([128, 512], f32, tag="stg")
                    nc.vector.tensor_copy(stg2[:, :], ps2[:, :])
                    nc.sync.dma_start(d["dB_sc"][:, imgbase * 256:(imgbase + 2) * 256], stg2[:, 0:512])

        # ============ PHASE B ============
        dBv = d["dB_cv1"][:, :].rearrange("p (i c) -> p i c", i=nimg, c=324)
        dBs = d["dB_sc"][:, :].rearrange("p (i c) -> p i c", i=nimg, c=256)
        for sbb in range(nB_):
            i0 = sbb * BSUB
            bufs = [act_tile(f"act{i}", BSUB * 324) for i in range(4)]
            for b in bufs:
                zero_r(b)
            res, tbuf, hbuf, t2buf = bufs
            scB = pact.tile([128, BSUB * 256], f32, tag="scb")
            nc.sync.dma_start(res[:].bitcast(f32).rearrange("p (i c) -> p i c", i=BSUB, c=324),
                              dBv[:, i0:i0 + BSUB, :])
            nc.sync.dma_start(scB[:].rearrange("p (i c) -> p i c", i=BSUB, c=256),
                              dBs[:, i0:i0 + BSUB, :])
            if ROUND_ALL:
                nc.vector.tensor_copy(res[:], res[:].bitcast(f32))

            def bview(t):
                return t[:].rearrange("p (i y x) -> p i y x", i=BSUB, y=18, x=18)

            rv, tv, hv, t2v = bview(res), bview(tbuf), bview(hbuf), bview(t2buf)
            scv = scB[:].rearrange("p (i y x) -> p i y x", i=BSUB, y=16, x=16)

            wc2 = load_w("wB0c2")
            for q in range(4):
                nc.scalar.activation(t2v[:, q * 4:q * 4 + 4, 1:17, 1:17], rv[:, q * 4:q * 4 + 4, 1:17, 1:17],
                                     Relu, bias=bnB("B0bn2"), scale=bnS("B0bn2"))
            for blk in range(BSUB // 2):
                ps = pps.tile([128, 512], f32, tag="cps")
                for j, (ky, kx) in enumerate(OFFS):
                    nc.tensor.matmul(
                        ps[:, :], wc2[:, j * 128:(j + 1) * 128],
                        t2v[:, blk * 2:blk * 2 + 2, ky:ky + 16, kx:kx + 16],
                        start=(j == 0), stop=(j == 8))
                pv = ps[:].rearrange("p (i y x) -> p i y x", i=2, y=16, x=16)
                nc.vector.tensor_add(tv[:, blk * 2:blk * 2 + 2, 1:17, 1:17], pv[:, :, :, :],
                                     scv[:, blk * 2:blk * 2 + 2, :, :])
            res, tbuf = tbuf, res
            rv, tv = tv, rv

            for bi in range(1, 4):
                wc1 = load_w(f"wB{bi}c1")
                wc2 = load_w(f"wB{bi}c2")
                for q in range(4):
                    nc.scalar.activation(tv[:, q * 4:q * 4 + 4, 1:17, 1:17], rv[:, q * 4:q * 4 + 4, 1:17, 1:17],
                                         Relu, bias=bnB(f"B{bi}bn1"), scale=bnS(f"B{bi}bn1"))
                for blk in range(BSUB // 2):
                    ps = pps.tile([128, 512], f32, tag="cps")
                    for j, (ky, kx) in enumerate(OFFS):
                        nc.tensor.matmul(
                            ps[:, :], wc1[:, j * 128:(j + 1) * 128],
                            tv[:, blk * 2:blk * 2 + 2, ky:ky + 16, kx:kx + 16],
                            start=(j == 0), stop=(j == 8))
                    pv = ps[:].rearrange("p (i y x) -> p i y x", i=2, y=16, x=16)
                    nc.scalar.copy(hv[:, blk * 2:blk * 2 + 2, 1:17, 1:17], pv[:, :, :, :])
                for q in range(4):
                    nc.scalar.activation(t2v[:, q * 4:q * 4 + 4, 1:17, 1:17], hv[:, q * 4:q * 4 + 4, 1:17, 1:17],
                                         Relu, bias=bnB(f"B{bi}bn2"), scale=bnS(f"B{bi}bn2"))
                for blk in range(BSUB // 2):
                    ps = pps.tile([128, 512], f32, tag="cps")
                    for j, (ky, kx) in enumerate(OFFS):
                        nc.tensor.matmul(
                            ps[:, :], wc2[:, j * 128:(j + 1) * 128],
                            t2v[:, blk * 2:blk * 2 + 2, ky:ky + 16, kx:kx + 16],
                            start=(j == 0), stop=(j == 8))
                    pv = ps[:].rearrange("p (i y x) -> p i y x", i=2, y=16, x=16)
                    nc.vector.tensor_add(tv[:, blk * 2:blk * 2 + 2, 1:17, 1:17], pv[:, :, :, :],
                                         rv[:, blk * 2:blk * 2 + 2, 1:17, 1:17])
                res, tbuf = tbuf, res
                rv, tv = tv, rv

            # transition B->C
            wt = load_w("wC0c1")
            wts = load_w("wC0sc")
            for q in range(4):
                nc.scalar.activation(tv[:, q * 4:q * 4 + 4, 1:17, 1:17], rv[:, q * 4:q * 4 + 4, 1:17, 1:17],
                                     Relu, bias=bnB("C0bn1"), scale=bnS("C0bn1"))
            dCv = d["dC_cv1"][:, :].rearrange("p (c n) -> p c n", c=2, n=nimg * 100)
            dCs = d["dC_sc"][:, :].rearrange("p (c n) -> p c n", c=2, n=nimg * 64)
            for blk8 in range(BSUB // 8):
                imgbase = i0 + blk8 * 8
                stg = pstg.tile([128, 1600], f32, tag="stg")
                nc.gpsimd.memset(stg[:], 0.0)
                sv = stg[:].rearrange("p (c i y x) -> p c i y x", c=2, i=8, y=10, x=10)
                stg2 = pstg.tile([128, 1024], f32, tag="stg")
                for mt in range(2):
                    ps = pps.tile([128, 512], f32, tag="cps")
                    for j, (ky, kx) in enumerate(OFFS):
                        nc.tensor.matmul(
                            ps[:, :], wt[:, j * 256 + mt * 128:j * 256 + mt * 128 + 128],
                            tv[:, blk8 * 8:blk8 * 8 + 8, ky:ky + 16:2, kx:kx + 16:2],
                            start=(j == 0), stop=(j == 8))
                    pv = ps[:].rearrange("p (i y x) -> p i y x", i=8, y=8, x=8)
                    nc.vector.tensor_copy(sv[:, mt, :, 1:9, 1:9], pv[:, :, :, :])
                    ps2 = pps.tile([128, 512], f32, tag="cps")
                    nc.tensor.matmul(
                        ps2[:, :], wts[:, mt * 128:mt * 128 + 128],
                        rv[:, blk8 * 8:blk8 * 8 + 8, 1:17:2, 1:17:2],
                        start=True, stop=True)
                    nc.vector.tensor_copy(stg2[:, mt * 512:(mt + 1) * 512], ps2[:, :])
                nc.sync.dma_start(dCv[:, :, imgbase * 100:(imgbase + 8) * 100],
                                  stg[:].rearrange("p (c n) -> p c n", c=2, n=800))
                nc.sync.dma_start(dCs[:, :, imgbase * 64:(imgbase + 8) * 64],
                                  stg2[:].rearrange("p (c n) -> p c n", c=2, n=512))

        # ============ PHASE C ============
        dCv = d["dC_cv1"][:, :].rearrange("p (c n) -> p c n", c=2, n=nimg * 100)
        dCs = d["dC_sc"][:, :].rearrange("p (c n) -> p c n", c=2, n=nimg * 64)
        for cb in range(nC):
            i0 = cb * CSUB
            bufs = [act_tile(f"act{i}", 2 * CSUB * 100) for i in range(4)]
            for b in bufs:
                zero_r(b)
            res, tbuf, hbuf, t2buf = bufs
            scC = pact.tile([128, 2 * CSUB * 64], f32, tag="scb")
            nc.sync.dma_start(res[:].bitcast(f32).rearrange("p (c n) -> p c n", c=2, n=CSUB * 100),
                              dCv[:, :, i0 * 100:(i0 + CSUB) * 100])
            nc.sync.dma_start(scC[:].rearrange("p (c n) -> p c n", c=2, n=CSUB * 64),
                              dCs[:, :, i0 * 64:(i0 + CSUB) * 64])
            if ROUND_ALL:
                nc.vector.tensor_copy(res[:], res[:].bitcast(f32))

            def cview(t):
                return t[:].rearrange("p (c i y x) -> p c i y x", c=2, i=CSUB, y=10, x=10)

            rv, tv, hv, t2v = cview(res), cview(tbuf), cview(hbuf), cview(t2buf)
            scv = scC[:].rearrange("p (c i y x) -> p c i y x", c=2, i=CSUB, y=8, x=8)
            nblk = CSUB // 8

            def conv_c(wtile, src, dst_assign):
                for blk in range(nblk):
                    pss = []
                    for mt in range(2):
                        ps = pps.tile([128, 512], f32, tag="cps")
                        k = 0
                        for c in range(2):
                            for j in range(9):
                                ky, kx = OFFS[j]
                                nc.tensor.matmul(
                                    ps[:, :],
                                    wtile[:, (c * 9 + j) * 256 + mt * 128:(c * 9 + j) * 256 + mt * 128 + 128],
                                    src[:, c, blk * 8:blk * 8 + 8, ky:ky + 8, kx:kx + 8],
                                    start=(k == 0), stop=(k == 17))
                                k += 1
                        pss.append(ps)
                    for mt in range(2):
                        pv = pss[mt][:].rearrange("p (i y x) -> p i y x", i=8, y=8, x=8)
                        dst_assign(mt, blk, pv)

            wc2 = load_w("wC0c2")
            for c in range(2):
                nc.scalar.activation(t2v[:, c, :, 1:9, 1:9], rv[:, c, :, 1:9, 1:9], Relu,
                                     bias=bnB("C0bn2", c), scale=bnS("C0bn2", c))

            def asg0(mt, blk, pv):
                nc.vector.tensor_add(tv[:, mt, blk * 8:blk * 8 + 8, 1:9, 1:9], pv[:, :, :, :],
                                     scv[:, mt, blk * 8:blk * 8 + 8, :, :])
            conv_c(wc2, t2v, asg0)
            res, tbuf = tbuf, res
            rv, tv = tv, rv

            for bi in range(1, 4):
                wc1 = load_w(f"wC{bi}c1")
                wc2 = load_w(f"wC{bi}c2")
                for c in range(2):
                    nc.scalar.activation(tv[:, c, :, 1:9, 1:9], rv[:, c, :, 1:9, 1:9], Relu,
                                         bias=bnB(f"C{bi}bn1", c), scale=bnS(f"C{bi}bn1", c))

                def asg1(mt, blk, pv):
                    nc.scalar.copy(hv[:, mt, blk * 8:blk * 8 + 8, 1:9, 1:9], pv[:, :, :, :])
                conv_c(wc1, tv, asg1)
                for c in range(2):
                    nc.scalar.activation(t2v[:, c, :, 1:9, 1:9], hv[:, c, :, 1:9, 1:9], Relu,
                                         bias=bnB(f"C{bi}bn2", c), scale=bnS(f"C{bi}bn2", c))

                def asg2(mt, blk, pv):
                    nc.vector.tensor_add(tv[:, mt, blk * 8:blk * 8 + 8, 1:9, 1:9], pv[:, :, :, :],
                                         rv[:, mt, blk * 8:blk * 8 + 8, 1:9, 1:9])
                conv_c(wc2, t2v, asg2)
                res, tbuf = tbuf, res
                rv, tv = tv, rv

            for c in range(2):
                nc.scalar.activation(tv[:, c, :, 1:9, 1:9], rv[:, c, :, 1:9, 1:9], Relu,
                                     bias=bnB("bnf", c), scale=bnS("bnf", c))
            tg = tbuf[:].rearrange("p (g y x) -> p g y x", g=2 * CSUB, y=10, x=10)
            fv = fT[:].rearrange("p (c i) -> p c i", c=2, i=nimg)
            nc.vector.reduce_sum(fv[:, :, i0:i0 + CSUB], tg[:, :, 1:9, 1:9], axis=AX.XY)

        # ============ PHASE D: gate + experts ============
        NI = nimg
        nc.vector.tensor_copy(fTr[:], fT[:])

        emr = load_w("emr")
        fc1w = load_w("fc1w")
        fc2w = load_w("fc2w")

        psG = ppst.tile([128, 512], f32, tag="tps")
        for c in range(2):
            nc.tensor.matmul(psG[:, 0:NI], emr[:, 90 + c * 128:90 + (c + 1) * 128],
                             fTr[:, c * NI:(c + 1) * NI], start=(c == 0), stop=(c == 1))
        gh = psml.tile([128, 64], f32r, tag="gh")
        nc.scalar.activation(gh[:, 0:NI], psG[:, 0:NI], Relu, bias=emf[:, 48:49], scale=1.0)
        psL = ppst.tile([128, 512], f32, tag="tps")
        nc.tensor.matmul(psL[0:8, 0:NI], emr[:, 346:354], gh[:, 0:NI], start=True, stop=True)
        logT = psml.tile([128, 64], f32, tag="logT")
        nc.scalar.activation(logT[0:8, 0:NI], psL[0:8, 0:NI], Iden, bias=emf[0:8, 49:50], scale=1.0)

        psT = ppst.tile([128, 512], f32, tag="tps")
        nc.tensor.transpose(psT[0:NI, 0:8], logT[0:8, 0:NI], ident[0:8, 0:8])
        l_sb = psml.tile([64, 8], f32, tag="lsb")
        nc.vector.tensor_copy(l_sb[0:NI, :], psT[0:NI, 0:8])

        m1 = psml.tile([64, 1], f32, tag="m1")
        m2 = psml.tile([64, 1], f32, tag="m2")
        msk1 = psml.tile([64, 8], f32, tag="msk1")
        msk2 = psml.tile([64, 8], f32, tag="msk2")
        l2 = psml.tile([64, 8], f32, tag="l2")
        dd = psml.tile([64, 1], f32, tag="dd")
        e2 = psml.tile([64, 1], f32, tag="e2")
        den = psml.tile([64, 1], f32, tag="den")
        rr = psml.tile([64, 1], f32, tag="rr")
        w2g = psml.tile([64, 1], f32, tag="w2g")
        g1t = psml.tile([64, 8], f32, tag="g1t")
        gates = psml.tile([64, 8], f32, tag="gates")
        nc.vector.reduce_max(m1[0:NI, :], l_sb[0:NI, :], axis=AX.X)
        nc.vector.tensor_scalar(msk1[0:NI, :], l_sb[0:NI, :], m1[0:NI, :], None, Alu.is_equal)
        nc.vector.scalar_tensor_tensor(l2[0:NI, :], msk1[0:NI, :], -1e30, l_sb[0:NI, :], Alu.mult, Alu.add)
        nc.vector.reduce_max(m2[0:NI, :], l2[0:NI, :], axis=AX.X)
        nc.vector.tensor_scalar(msk2[0:NI, :], l2[0:NI, :], m2[0:NI, :], None, Alu.is_equal)
        nc.vector.tensor_sub(dd[0:NI, :], m2[0:NI, :], m1[0:NI, :])
        nc.scalar.activation(e2[0:NI, :], dd[0:NI, :], Exp)
        nc.vector.tensor_scalar_add(den[0:NI, :], e2[0:NI, :], 1.0)
        nc.vector.reciprocal(rr[0:NI, :], den[0:NI, :])
        nc.vector.tensor_mul(w2g[0:NI, :], e2[0:NI, :], rr[0:NI, :])
        nc.vector.tensor_scalar(g1t[0:NI, :], msk1[0:NI, :], rr[0:NI, :], None, Alu.mult)
        nc.vector.scalar_tensor_tensor(gates[0:NI, :], msk2[0:NI, :], w2g[0:NI, :], g1t[0:NI, :],
                                       Alu.mult, Alu.add)

        psT2 = ppst.tile([128, 512], f32, tag="tps")
        nc.tensor.transpose(psT2[0:8, 0:NI], gates[0:NI, 0:8], ident[0:NI, 0:NI])
        gTr = psml.tile([128, 64], f32r, tag="gTr")
        nc.vector.tensor_copy(gTr[0:8, 0:NI], psT2[0:8, 0:NI])

        h1 = psml.tile([128, 128], f32r, tag="h1")
        h2 = psml.tile([128, 64], f32r, tag="h2")
        accA = psml.tile([64, 10], f32, tag="accA")
        accB = psml.tile([64, 10], f32, tag="accB")

        psb = ppse.tile([128, 512], f32, tag="eps")
        nc.tensor.matmul(psb[0:NI, 0:10], gTr[0:8, 0:NI], emr[0:8, 80:90], start=True, stop=True)
        nc.vector.tensor_copy(accA[0:NI, :], psb[0:NI, 0:10])
        src, dst = accA, accB
        for e in range(8):
            for mt in range(2):
                ps1 = ppse.tile([128, 512], f32, tag="eps")
                for c in range(2):
                    nc.tensor.matmul(ps1[:, 0:NI],
                                     fc1w[:, ((e * 2 + c) * 2 + mt) * 128:((e * 2 + c) * 2 + mt + 1) * 128],
                                     fTr[:, c * NI:(c + 1) * NI], start=(c == 0), stop=(c == 1))
                nc.scalar.activation(h1[:, mt * 64:mt * 64 + NI], ps1[:, 0:NI], Relu,
                                     bias=emf[:, 16 + e * 2 + mt:17 + e * 2 + mt],
                                     scale=emf[:, e * 2 + mt:e * 2 + mt + 1])
            ps2 = ppse.tile([128, 512], f32, tag="eps")
            for hc in range(2):
                nc.tensor.matmul(ps2[:, 0:NI], fc2w[:, (e * 2 + hc) * 128:(e * 2 + hc + 1) * 128],
                                 h1[:, hc * 64:hc * 64 + NI], start=(hc == 0), stop=(hc == 1))
            nc.scalar.activation(h2[:, 0:NI], ps2[:, 0:NI], Relu,
                                 bias=emf[:, 40 + e:41 + e], scale=emf[:, 32 + e:33 + e])
            ps3 = ppse.tile([128, 512], f32, tag="eps")
            nc.tensor.matmul(ps3[0:NI, 0:10], h2[:, 0:NI], emr[:, e * 10:(e + 1) * 10],
                             start=True, stop=True)
            nc.vector.scalar_tensor_tensor(dst[0:NI, :], ps3[0:NI, 0:10], gates[0:NI, e:e + 1],
                                           src[0:NI, :], Alu.mult, Alu.add)
            src, dst = dst, src

        nc.sync.dma_start(d["out"][:, :], src[0:NI, :])

    nc.compile()
    return nc


# ---------------------------------------------------------------------------
# Public entry point
# ---------------------------------------------------------------------------

_CACHE = {}


def kernel(x, params):
    from concourse.bass_utils import run_bass_kernel_spmd

    x = np.asarray(x, dtype=np.float32)
    B = x.shape[0]
    per = B // N_CORES
    assert per == 64, f"expected 64 imgs/core, got {per}"

    if "nc" not in _CACHE:
        shared, colmap = prep_shared(params)
        nc = build_nc(colmap, nimg=64)
        _CACHE["nc"] = (nc, shared)
    nc, shared = _CACHE["nc"]

    in_maps = []
    for c in range(N_CORES):
        m = dict(shared)
        m["xcol"] = prep_xcol(x[c * per:(c + 1) * per])
        in_maps.append(m)

    res = run_bass_kernel_spmd(nc, in_maps, core_ids=list(range(N_CORES)))
    out = np.concatenate([res.results[c]["out"] for c in range(N_CORES)], axis=0)
    return out.astype(np.float32)


# revision 18
# speedup vs baseline: 9.7065x; 9.7065x over previous
# Trainium2 Bass kernel for nn_Net_89687507075560 (ResNet feature extractor + top-2 MoE head).
# Data-parallel over batch: 512 images -> 8 cores x 64 images, params replicated.
# Convs run as fp32r matmuls (full-rate, ~12-bit-mantissa fp32) with fp32 PSUM accumulation.
# Stage A (32x32, C<=64) uses 2-group row/col array tiling (imgs split across partition halves).
# Activations spill to DRAM between stages (SBUF working set per stage stays < 200KB/partition).
import sys

sys.path.insert(0, "/opt/trn_rl_repo")

import numpy as np

N_CORES = 8
EPS = 1e-5
ROUND_ALL = False  # add explicit fp32r rounding copies after memset/DMA producers

# ---------------------------------------------------------------------------
# Host-side weight preparation
# ---------------------------------------------------------------------------

def _np(a):
    return np.asarray(a, dtype=np.float32)


def _fold_bn(p):
    g, b, m, v = _np(p["g"]), _np(p["b"]), _np(p["m"]), _np(p["v"])
    inv = g / np.sqrt(v + EPS)
    return inv, b - m * inv


def _conv_w_tile(W, replicate):
    O, I = W.shape[0], W.shape[1]
    out = np.zeros((128, 9 * O), dtype=np.float32)
    for ky in range(3):
        for kx in range(3):
            j = ky * 3 + kx
            wj = W[:, :, ky, kx].T
            out[:I, j * O:(j + 1) * O] = wj
            if replicate:
                out[64:64 + I, j * O:(j + 1) * O] = wj
    return out


def _conv_w_tile_c(W):
    O = W.shape[0]
    out = np.zeros((128, 2 * 9 * O), dtype=np.float32)
    for c in range(2):
        for ky in range(3):
            for kx in range(3):
                j = ky * 3 + kx
                out[:, (c * 9 + j) * O:(c * 9 + j + 1) * O] = W[:, c * 128:(c + 1) * 128, ky, kx].T
    return out


def _sc_w_tile(W, replicate):
    O, I = W.shape[0], W.shape[1]
    out = np.zeros((128, O), dtype=np.float32)
    out[:I, :] = W[:, :, 0, 0].T
    if replicate:
        out[64:64 + I, :] = W[:, :, 0, 0].T
    return out


def _bn_cols(bnv, colmap, key, inv, bias, replicate, nchunk=1):
    C = inv.shape[0]
    if nchunk == 1:
        s = np.zeros(128, np.float32)
        b = np.zeros(128, np.float32)
        s[:C] = inv
        b[:C] = bias
        if replicate:
            s[64:64 + C] = inv
            b[64:64 + C] = bias
        colmap[key] = (len(bnv), 1)
        bnv.append(s)
        bnv.append(b)
    else:
        colmap[key] = (len(bnv), 2)
        bnv.append(inv[:128])
        bnv.append(inv[128:])
        bnv.append(bias[:128])
        bnv.append(bias[128:])


def prep_shared(params):
    arrs = {}
    p = params
    arrs["w27"] = _np(p["conv1"]).transpose(2, 3, 1, 0).reshape(27, 16).copy()

    bnv = []
    colmap = {}

    for bi, bp in enumerate(p["block1"]):
        inv1, b1 = _fold_bn(bp["bn1"])
        inv2, b2 = _fold_bn(bp["bn2"])
        _bn_cols(bnv, colmap, f"A{bi}bn1", inv1, b1, replicate=True)
        _bn_cols(bnv, colmap, f"A{bi}bn2", inv2, b2, replicate=True)
        arrs[f"wA{bi}c1"] = _conv_w_tile(_np(bp["conv1"]), replicate=True)
        arrs[f"wA{bi}c2"] = _conv_w_tile(_np(bp["conv2"]), replicate=True)
        if "shortcut" in bp:
            arrs[f"wA{bi}sc"] = _sc_w_tile(_np(bp["shortcut"]), replicate=True)
    for bi, bp in enumerate(p["block2"]):
        inv1, b1 = _fold_bn(bp["bn1"])
        inv2, b2 = _fold_bn(bp["bn2"])
        _bn_cols(bnv, colmap, f"B{bi}bn1", inv1, b1, replicate=(bi == 0))
        _bn_cols(bnv, colmap, f"B{bi}bn2", inv2, b2, replicate=False)
        arrs[f"wB{bi}c1"] = _conv_w_tile(_np(bp["conv1"]), replicate=(bi == 0))
        arrs[f"wB{bi}c2"] = _conv_w_tile(_np(bp["conv2"]), replicate=False)
        if "shortcut" in bp:
            arrs[f"wB{bi}sc"] = _sc_w_tile(_np(bp["shortcut"]), replicate=(bi == 0))
    for bi, bp in enumerate(p["block3"]):
        inv1, b1 = _fold_bn(bp["bn1"])
        inv2, b2 = _fold_bn(bp["bn2"])
        _bn_cols(bnv, colmap, f"C{bi}bn1", inv1, b1, replicate=False, nchunk=1 if bi == 0 else 2)
        _bn_cols(bnv, colmap, f"C{bi}bn2", inv2, b2, replicate=False, nchunk=2)
        if bi == 0:
            arrs[f"wC{bi}c1"] = _conv_w_tile(_np(bp["conv1"]), replicate=False)
        else:
            arrs[f"wC{bi}c1"] = _conv_w_tile_c(_np(bp["conv1"]))
        arrs[f"wC{bi}c2"] = _conv_w_tile_c(_np(bp["conv2"]))
        if "shortcut" in bp:
            arrs[f"wC{bi}sc"] = _sc_w_tile(_np(bp["shortcut"]), replicate=False)
    invf, bf = _fold_bn(p["bn_final"])
    _bn_cols(bnv, colmap, "bnf", invf / 64.0, bf / 64.0, replicate=False, nchunk=2)

    bn_arr = np.zeros((128, len(bnv)), dtype=np.float32)
    for i, col in enumerate(bnv):
        bn_arr[:, i] = col
    arrs["bnv"] = bn_arr

    arrs["ident"] = np.eye(128, dtype=np.float32)

    gp = p["gate"]
    T = float(np.clip(_np(gp["temperature"])[0], 0.5, 2.0))
    w1, gb1 = _np(gp["w1"]), _np(gp["b1"])
    w2, gb2 = _np(gp["w2"]) / T, _np(gp["b2"]) / T
    ep = p["experts"]
    fc1w = _np(ep["fc1_w"])
    fc1b = _np(ep["fc1_b"])
    inv1e = np.zeros((8, 256), np.float32)
    bia1e = np.zeros((8, 256), np.float32)
    g_, b_, m_, v_ = _np(ep["bn1"]["g"]), _np(ep["bn1"]["b"]), _np(ep["bn1"]["m"]), _np(ep["bn1"]["v"])
    for e in range(8):
        iv = g_[e] / np.sqrt(v_[e] + EPS)
        inv1e[e] = iv
        bia1e[e] = fc1b[e] * iv + (b_[e] - m_[e] * iv)
    fc2w = _np(ep["fc2_w"])
    fc2b = _np(ep["fc2_b"])
    inv2e = np.zeros((8, 128), np.float32)
    bia2e = np.zeros((8, 128), np.float32)
    g_, b_, m_, v_ = _np(ep["bn2"]["g"]), _np(ep["bn2"]["b"]), _np(ep["bn2"]["m"]), _np(ep["bn2"]["v"])
    for e in range(8):
        iv = g_[e] / np.sqrt(v_[e] + EPS)
        inv2e[e] = iv
        bia2e[e] = fc2b[e] * iv + (b_[e] - m_[e] * iv)
    fc3w = _np(ep["fc3_w"])
    fc3b = _np(ep["fc3_b"])

    a = np.zeros((128, 8 * 2 * 2 * 128), np.float32)
    for e in range(8):
        for c in range(2):
            for mt in range(2):
                a[:, ((e * 2 + c) * 2 + mt) * 128:((e * 2 + c) * 2 + mt + 1) * 128] = \
                    fc1w[e, c * 128:(c + 1) * 128, mt * 128:(mt + 1) * 128]
    arrs["fc1w"] = a
    a = np.zeros((128, 8 * 2 * 128), np.float32)
    for e in range(8):
        for hc in range(2):
            a[:, (e * 2 + hc) * 128:(e * 2 + hc + 1) * 128] = fc2w[e, hc * 128:(hc + 1) * 128, :]
    arrs["fc2w"] = a
    emr = np.zeros((128, 354), np.float32)
    for e in range(8):
        emr[:, e * 10:(e + 1) * 10] = fc3w[e]
    emr[:8, 80:90] = fc3b
    for c in range(2):
        emr[:, 90 + c * 128:90 + (c + 1) * 128] = w1[c * 128:(c + 1) * 128, :]
    emr[:, 346:354] = w2
    arrs["emr"] = emr
    emf = np.zeros((128, 50), np.float32)
    for e in range(8):
        for mt in range(2):
            emf[:, e * 2 + mt] = inv1e[e, mt * 128:(mt + 1) * 128]
            emf[:, 16 + e * 2 + mt] = bia1e[e, mt * 128:(mt + 1) * 128]
        emf[:, 32 + e] = inv2e[e]
        emf[:, 40 + e] = bia2e[e]
    emf[:, 48] = gb1
    emf[:8, 49] = gb2
    arrs["emf"] = emf

    return arrs, colmap


def prep_xcol(x_core):
    NI = x_core.shape[0]
    xp = np.zeros((NI, 3, 34, 34), dtype=np.float32)
    xp[:, :, 1:33, 1:33] = x_core
    out = np.empty((27, NI * 1024), dtype=np.float32)
    for ky in range(3):
        for kx in range(3):
            j = ky * 3 + kx
            out[j * 3:(j + 1) * 3, :] = (
                xp[:, :, ky:ky + 32, kx:kx + 32].transpose(1, 0, 2, 3).reshape(3, NI * 1024)
            )
    return out


# ---------------------------------------------------------------------------
# Kernel emission
# ---------------------------------------------------------------------------

OFFS = [(ky, kx) for ky in range(3) for kx in range(3)]


def build_nc(colmap, nimg=64, reps=None, dummy_inputs=False):
    import concourse.bacc as bacc
    import concourse.tile as tile
    import concourse.mybir as mybir
    from concourse.alu_op_type import AluOpType as Alu
    import bass_rust
    from contextlib import ExitStack, nullcontext

    f32 = mybir.dt.float32
    f32r = mybir.dt.float32r
    Relu = mybir.ActivationFunctionType.Relu
    Iden = mybir.ActivationFunctionType.Identity
    Exp = mybir.ActivationFunctionType.Exp
    AX = bass_rust.AxisListType

    nc = bacc.Bacc("TRN2", target_bir_lowering=False, debug=False, num_devices=N_CORES)

    shapes = {
        "w27": [27, 16], "ident": [128, 128],
        "fc1w": [128, 4096], "fc2w": [128, 2048], "emr": [128, 354], "emf": [128, 50],
        "xcol": [27, nimg * 1024],
    }
    wshapes = {}
    for bi in range(4):
        wshapes[f"wA{bi}c1"] = [128, 576]
        wshapes[f"wA{bi}c2"] = [128, 576]
    wshapes["wA0sc"] = [128, 64]
    wshapes["wB0sc"] = [128, 128]
    for bi in range(4):
        wshapes[f"wB{bi}c1"] = [128, 1152]
        wshapes[f"wB{bi}c2"] = [128, 1152]
    wshapes["wC0c1"] = [128, 2304]
    wshapes["wC0sc"] = [128, 256]
    wshapes["wC0c2"] = [128, 4608]
    for bi in range(1, 4):
        wshapes[f"wC{bi}c1"] = [128, 4608]
        wshapes[f"wC{bi}c2"] = [128, 4608]

    NB = 2 * sum(v[1] for v in colmap.values())
    shapes["bnv"] = [128, NB]

    d = {}
    rnd_names = set(wshapes) | {"fc1w", "fc2w", "emr"}
    for name, shp in {**shapes, **wshapes}.items():
        dt_ = f32r if name in rnd_names else f32
        if dummy_inputs:
            d[name] = nc.dram_tensor(name, shp, dt_)
        else:
            d[name] = nc.dram_tensor(name, shp, dt_, kind="ExternalInput")
    d["out"] = nc.dram_tensor("out", [nimg, 10], f32, kind="ExternalOutput")
    d["dB_cv1"] = nc.dram_tensor("dB_cv1", [128, nimg * 324], f32)
    d["dB_sc"] = nc.dram_tensor("dB_sc", [128, nimg * 256], f32)
    d["dC_cv1"] = nc.dram_tensor("dC_cv1", [128, 2 * nimg * 100], f32)
    d["dC_sc"] = nc.dram_tensor("dC_sc", [128, 2 * nimg * 64], f32)

    ASUB, BSUB, CSUB = 8, 16, 16
    nA, nB_, nC = nimg // ASUB, nimg // BSUB, nimg // CSUB

    with tile.TileContext(nc) as tc, ExitStack() as ctx:
        pact = ctx.enter_context(tc.tile_pool(name="acts", bufs=1))
        psml = ctx.enter_context(tc.tile_pool(name="small", bufs=1))
        pw = ctx.enter_context(tc.tile_pool(name="wst", bufs=3))
        pstg = ctx.enter_context(tc.tile_pool(name="stg", bufs=3))
        pxc = ctx.enter_context(tc.tile_pool(name="xc", bufs=2))
        pps = ctx.enter_context(tc.tile_pool(name="cps", bufs=4, space="PSUM"))
        ppse = ctx.enter_context(tc.tile_pool(name="eps", bufs=2, space="PSUM"))
        ppst = ctx.enter_context(tc.tile_pool(name="tps", bufs=2, space="PSUM"))

        bnv = psml.tile([128, NB], f32, tag="bnv")
        nc.sync.dma_start(bnv[:], d["bnv"][:, :])
        ident = psml.tile([128, 128], f32, tag="ident")
        nc.sync.dma_start(ident[:], d["ident"][:, :])
        emf = psml.tile([128, 50], f32, tag="emf")
        nc.sync.dma_start(emf[:], d["emf"][:, :])
        fT = psml.tile([128, 2 * nimg], f32, tag="fT")
        fTr = psml.tile([128, 2 * nimg], f32r, tag="fTr")
        ZC = max(4 * 1156, BSUB * 324, 2 * CSUB * 100) // 2
        zeros = psml.tile([128, ZC], f32, tag="zeros")
        nc.gpsimd.memset(zeros[:], 0.0)

        def bnS(key, c=0):
            col, nch = colmap[key]
            return bnv[:, col + c:col + c + 1]

        def bnB(key, c=0):
            col, nch = colmap[key]
            return bnv[:, col + nch + c:col + nch + c + 1]

        def load_w(name, rnd=True):
            shp = wshapes.get(name) or shapes[name]
            t = pw.tile([128, shp[1]], f32r if rnd else f32, tag="wst")
            nc.sync.dma_start(t[0:shp[0], :], d[name][:, :])
            return t

        def act_tile(tag, cols):
            t = pact.tile([128, cols], f32r, tag=tag)
            return t

        def zero_r(t):
            # f32r memset is invalid ISA; produce zeros via DVE copy (a "rounding" op)
            half = t.shape[1] // 2
            nc.vector.tensor_copy(t[:, 0:half], zeros[:, 0:half])
            nc.vector.tensor_copy(t[:, half:t.shape[1]], zeros[:, 0:t.shape[1] - half])

        if reps is not None:
            ctx.enter_context(tc.For_i(0, reps, 1))

        # ============ PHASE A ============
        for sb in range(nA):
            bufs = [act_tile(f"act{i}", 4 * 1156) for i in range(4)]
            for b in bufs:
                zero_r(b)
            res, tbuf, hbuf, t2buf = bufs

            def aview(t):
                return t[:].rearrange("p (i y x) -> p i y x", i=4, y=34, x=34)

            rv, tv, hv, t2v = aview(res), aview(tbuf), aview(hbuf), aview(t2buf)

            # conv1 (3->16), plain fp32, both groups via psum col split
            w27 = load_w("w27", rnd=False)
            xv = d["xcol"][:, :].rearrange("p (i n) -> p i n", i=nimg, n=1024)
            for il in range(4):
                xc = pxc.tile([27, 2048], f32, tag="xc")
                g1 = sb * ASUB + il
                g2 = sb * ASUB + 4 + il
                nc.sync.dma_start(xc[:, 0:1024], xv[:, g1, :])
                nc.sync.dma_start(xc[:, 1024:2048], xv[:, g2, :])
                for h in range(2):
                    psA = pps.tile([128, 512], f32, tag="cps")
                    psB = pps.tile([128, 512], f32, tag="cps")
                    nc.tensor.matmul(psA[0:16, :], w27[0:27, :], xc[0:27, h * 512:(h + 1) * 512],
                                     start=True, stop=True)
                    nc.tensor.matmul(psB[0:16, :], w27[0:27, :], xc[0:27, 1024 + h * 512:1536 + h * 512],
                                     start=True, stop=True)
                    pvA = psA[:].rearrange("p (y x) -> p y x", y=16, x=32)
                    pvB = psB[:].rearrange("p (y x) -> p y x", y=16, x=32)
                    nc.scalar.copy(rv[0:16, il, 1 + 16 * h:17 + 16 * h, 1:33], pvA[0:16, :, :])
                    nc.scalar.copy(rv[64:80, il, 1 + 16 * h:17 + 16 * h, 1:33], pvB[0:16, :, :])

            for bi in range(4):
                Ic = 16 if bi == 0 else 64
                wc1 = load_w(f"wA{bi}c1")
                wc2 = load_w(f"wA{bi}c2")
                wsc = load_w("wA0sc") if bi == 0 else None

                for il in range(4):
                    nc.scalar.activation(tv[:, il, 1:33, 1:33], rv[:, il, 1:33, 1:33], Relu,
                                         bias=bnB(f"A{bi}bn1"), scale=bnS(f"A{bi}bn1"))
                for il in range(4):
                    for h in range(2):
                        psA = pps.tile([128, 512], f32, tag="cps")
                        psB = pps.tile([128, 512], f32, tag="cps")
                        for j, (ky, kx) in enumerate(OFFS):
                            for pb, ps in ((0, psA), (64, psB)):
                                nc.tensor.matmul(
                                    ps[0:64, :],
                                    wc1[pb:pb + Ic, j * 64:(j + 1) * 64],
                                    tv[pb:pb + Ic, il, 16 * h + ky:16 * h + ky + 16, kx:kx + 32],
                                    start=(j == 0), stop=(j == 8))
                        pvA = psA[:].rearrange("p (y x) -> p y x", y=16, x=32)
                        pvB = psB[:].rearrange("p (y x) -> p y x", y=16, x=32)
                        nc.scalar.copy(hv[0:64, il, 1 + 16 * h:17 + 16 * h, 1:33], pvA[0:64, :, :])
                        nc.scalar.copy(hv[64:128, il, 1 + 16 * h:17 + 16 * h, 1:33], pvB[0:64, :, :])
                for il in range(4):
                    nc.scalar.activation(t2v[:, il, 1:33, 1:33], hv[:, il, 1:33, 1:33], Relu,
                                         bias=bnB(f"A{bi}bn2"), scale=bnS(f"A{bi}bn2"))
                for il in range(4):
                    for h in range(2):
                        psA = pps.tile([128, 512], f32, tag="cps")
                        psB = pps.tile([128, 512], f32, tag="cps")
                        for j, (ky, kx) in enumerate(OFFS):
                            for pb, ps in ((0, psA), (64, psB)):
                                nc.tensor.matmul(
                                    ps[0:64, :],
                                    wc2[pb:pb + 64, j * 64:(j + 1) * 64],
                                    t2v[pb:pb + 64, il, 16 * h + ky:16 * h + ky + 16, kx:kx + 32],
                                    start=(j == 0), stop=(j == 8 and wsc is None))
                        if wsc is not None:
                            for pb, ps in ((0, psA), (64, psB)):
                                nc.tensor.matmul(
                                    ps[0:64, :],
                                    wsc[pb:pb + Ic, 0:64],
                                    rv[pb:pb + Ic, il, 1 + 16 * h:17 + 16 * h, 1:33],
                                    start=False, stop=True)
                        for pb, ps in ((0, psA), (64, psB)):
                            pv = ps[:].rearrange("p (y x) -> p y x", y=16, x=32)
                            dst = tv[pb:pb + 64, il, 1 + 16 * h:17 + 16 * h, 1:33]
                            if wsc is not None:
                                nc.vector.tensor_copy(dst, pv[0:64, :, :])
                            else:
                                nc.vector.tensor_add(dst, pv[0:64, :, :],
                                                     rv[pb:pb + 64, il, 1 + 16 * h:17 + 16 * h, 1:33])
                res, tbuf = tbuf, res
                rv, tv = tv, rv

            # transition A->B: t = relu(bn1_B0(res)); cv1 = conv_s2(t); sc = conv1x1_s2(res)
            wt = load_w("wB0c1")
            wts = load_w("wB0sc")
            for il in range(4):
                nc.scalar.activation(tv[:, il, 1:33, 1:33], rv[:, il, 1:33, 1:33], Relu,
                                     bias=bnB("B0bn1"), scale=bnS("B0bn1"))
            for grp in range(2):
                pb = grp * 64
                for il2 in range(2):
                    imgbase = sb * ASUB + grp * 4 + il2 * 2
                    ps = pps.tile([128, 512], f32, tag="cps")
                    for j, (ky, kx) in enumerate(OFFS):
                        nc.tensor.matmul(
                            ps[:, :],
                            wt[pb:pb + 64, j * 128:(j + 1) * 128],
                            tv[pb:pb + 64, il2 * 2:il2 * 2 + 2, ky:ky + 32:2, kx:kx + 32:2],
                            start=(j == 0), stop=(j == 8))
                    stg = pstg.tile([128, 648], f32, tag="stg")
                    nc.gpsimd.memset(stg[:], 0.0)
                    sv = stg[:].rearrange("p (i y x) -> p i y x", i=2, y=18, x=18)
                    pv = ps[:].rearrange("p (i y x) -> p i y x", i=2, y=16, x=16)
                    nc.vector.tensor_copy(sv[:, :, 1:17, 1:17], pv[:, :, :, :])
                    nc.sync.dma_start(d["dB_cv1"][:, imgbase * 324:(imgbase + 2) * 324], stg[:])
                    ps2 = pps.tile([128, 512], f32, tag="cps")
                    nc.tensor.matmul(
                        ps2[:, :],
                        wts[pb:pb + 64, 0:128],
                        rv[pb:pb + 64, il2 * 2:il2 * 2 + 2, 1:33:2, 1:33:2],
                        start=True, stop=True)
                    stg2 = pstg.tile([128, 512], f32, tag="stg")
                    nc.vector.tensor_copy(stg2[:, :], ps2[:, :])
                    nc.sync.dma_start(d["dB_sc"][:, imgbase * 256:(imgbase + 2) * 256], stg2[:, 0:512])

        # ============ PHASE B ============
        dBv = d["dB_cv1"][:, :].rearrange("p (i c) -> p i c", i=nimg, c=324)
        dBs = d["dB_sc"][:, :].rearrange("p (i c) -> p i c", i=nimg, c=256)
        for sbb in range(nB_):
            i0 = sbb * BSUB
            bufs = [act_tile(f"act{i}", BSUB * 324) for i in range(4)]
            for b in bufs:
                zero_r(b)
            res, tbuf, hbuf, t2buf = bufs
            scB = pact.tile([128, BSUB * 256], f32, tag="scb")
            nc.sync.dma_start(res[:].bitcast(f32).rearrange("p (i c) -> p i c", i=BSUB, c=324),
                              dBv[:, i0:i0 + BSUB, :])
            nc.sync.dma_start(scB[:].rearrange("p (i c) -> p i c", i=BSUB, c=256),
                              dBs[:, i0:i0 + BSUB, :])
            if ROUND_ALL:
                nc.vector.tensor_copy(res[:], res[:].bitcast(f32))

            def bview(t):
                return t[:].rearrange("p (i y x) -> p i y x", i=BSUB, y=18, x=18)

            rv, tv, hv, t2v = bview(res), bview(tbuf), bview(hbuf), bview(t2buf)
            scv = scB[:].rearrange("p (i y x) -> p i y x", i=BSUB, y=16, x=16)

            wc2 = load_w("wB0c2")
            for q in range(4):
                nc.scalar.activation(t2v[:, q * 4:q * 4 + 4, 1:17, 1:17], rv[:, q * 4:q * 4 + 4, 1:17, 1:17],
                                     Relu, bias=bnB("B0bn2"), scale=bnS("B0bn2"))
            for blk in range(BSUB // 2):
                ps = pps.tile([128, 512], f32, tag="cps")
                for j, (ky, kx) in enumerate(OFFS):
                    nc.tensor.matmul(
                        ps[:, :], wc2[:, j * 128:(j + 1) * 128],
                        t2v[:, blk * 2:blk * 2 + 2, ky:ky + 16, kx:kx + 16],
                        start=(j == 0), stop=(j == 8))
                pv = ps[:].rearrange("p (i y x) -> p i y x", i=2, y=16, x=16)
                nc.vector.tensor_add(tv[:, blk * 2:blk * 2 + 2, 1:17, 1:17], pv[:, :, :, :],
                                     scv[:, blk * 2:blk * 2 + 2, :, :])
            res, tbuf = tbuf, res
            rv, tv = tv, rv

            for bi in range(1, 4):
                wc1 = load_w(f"wB{bi}c1")
                wc2 = load_w(f"wB{bi}c2")
                for q in range(4):
                    nc.scalar.activation(tv[:, q * 4:q * 4 + 4, 1:17, 1:17], rv[:, q * 4:q * 4 + 4, 1:17, 1:17],
                                         Relu, bias=bnB(f"B{bi}bn1"), scale=bnS(f"B{bi}bn1"))
                for blk in range(BSUB // 2):
                    ps = pps.tile([128, 512], f32, tag="cps")
                    for j, (ky, kx) in enumerate(OFFS):
                        nc.tensor.matmul(
                            ps[:, :], wc1[:, j * 128:(j + 1) * 128],
                            tv[:, blk * 2:blk * 2 + 2, ky:ky + 16, kx:kx + 16],
                            start=(j == 0), stop=(j == 8))
                    pv = ps[:].rearrange("p (i y x) -> p i y x", i=2, y=16, x=16)
                    nc.scalar.copy(hv[:, blk * 2:blk * 2 + 2, 1:17, 1:17], pv[:, :, :, :])
                for q in range(4):
                    nc.scalar.activation(t2v[:, q * 4:q * 4 + 4, 1:17, 1:17], hv[:, q * 4:q * 4 + 4, 1:17, 1:17],
                                         Relu, bias=bnB(f"B{bi}bn2"), scale=bnS(f"B{bi}bn2"))
                for blk in range(BSUB // 2):
                    ps = pps.tile([128, 512], f32, tag="cps")
                    for j, (ky, kx) in enumerate(OFFS):
                        nc.tensor.matmul(
                            ps[:, :], wc2[:, j * 128:(j + 1) * 128],
                            t2v[:, blk * 2:blk * 2 + 2, ky:ky + 16, kx:kx + 16],
                            start=(j == 0), stop=(j == 8))
                    pv = ps[:].rearrange("p (i y x) -> p i y x", i=2, y=16, x=16)
                    nc.vector.tensor_add(tv[:, blk * 2:blk * 2 + 2, 1:17, 1:17], pv[:, :, :, :],
                                         rv[:, blk * 2:blk * 2 + 2, 1:17, 1:17])
                res, tbuf = tbuf, res
                rv, tv = tv, rv

            # transition B->C
            wt = load_w("wC0c1")
            wts = load_w("wC0sc")
            for q in range(4):
                nc.scalar.activation(tv[:, q * 4:q * 4 + 4, 1:17, 1:17], rv[:, q * 4:q * 4 + 4, 1:17, 1:17],
                                     Relu, bias=bnB("C0bn1"), scale=bnS("C0bn1"))
            dCv = d["dC_cv1"][:, :].rearrange("p (c n) -> p c n", c=2, n=nimg * 100)
            dCs = d["dC_sc"][:, :].rearrange("p (c n) -> p c n", c=2, n=nimg * 64)
            for blk8 in range(BSUB // 8):
                imgbase = i0 + blk8 * 8
                stg = pstg.tile([128, 1600], f32, tag="stg")
                nc.gpsimd.memset(stg[:], 0.0)
                sv = stg[:].rearrange("p (c i y x) -> p c i y x", c=2, i=8, y=10, x=10)
                stg2 = pstg.tile([128, 1024], f32, tag="stg")
                for mt in range(2):
                    ps = pps.tile([128, 512], f32, tag="cps")
                    for j, (ky, kx) in enumerate(OFFS):
                        nc.tensor.matmul(
                            ps[:, :], wt[:, j * 256 + mt * 128:j * 256 + mt * 128 + 128],
                            tv[:, blk8 * 8:blk8 * 8 + 8, ky:ky + 16:2, kx:kx + 16:2],
                            start=(j == 0), stop=(j == 8))
                    pv = ps[:].rearrange("p (i y x) -> p i y x", i=8, y=8, x=8)
                    nc.vector.tensor_copy(sv[:, mt, :, 1:9, 1:9], pv[:, :, :, :])
                    ps2 = pps.tile([128, 512], f32, tag="cps")
                    nc.tensor.matmul(
                        ps2[:, :], wts[:, mt * 128:mt * 128 + 128],
                        rv[:, blk8 * 8:blk8 * 8 + 8, 1:17:2, 1:17:2],
                        start=True, stop=True)
                    nc.vector.tensor_copy(stg2[:, mt * 512:(mt + 1) * 512], ps2[:, :])
                nc.sync.dma_start(dCv[:, :, imgbase * 100:(imgbase + 8) * 100],
                                  stg[:].rearrange("p (c n) -> p c n", c=2, n=800))
                nc.sync.dma_start(dCs[:, :, imgbase * 64:(imgbase + 8) * 64],
                                  stg2[:].rearrange("p (c n) -> p c n", c=2, n=512))

        # ============ PHASE C ============
        dCv = d["dC_cv1"][:, :].rearrange("p (c n) -> p c n", c=2, n=nimg * 100)
        dCs = d["dC_sc"][:, :].rearrange("p (c n) -> p c n", c=2, n=nimg * 64)
        for cb in range(nC):
            i0 = cb * CSUB
            bufs = [act_tile(f"act{i}", 2 * CSUB * 100) for i in range(4)]
            for b in bufs:
                zero_r(b)
            res, tbuf, hbuf, t2buf = bufs
            scC = pact.tile([128, 2 * CSUB * 64], f32, tag="scb")
            nc.sync.dma_start(res[:].bitcast(f32).rearrange("p (c n) -> p c n", c=2, n=CSUB * 100),
                              dCv[:, :, i0 * 100:(i0 + CSUB) * 100])
            nc.sync.dma_start(scC[:].rearrange("p (c n) -> p c n", c=2, n=CSUB * 64),
                              dCs[:, :, i0 * 64:(i0 + CSUB) * 64])
            if ROUND_ALL:
                nc.vector.tensor_copy(res[:], res[:].bitcast(f32))

            def cview(t):
                return t[:].rearrange("p (c i y x) -> p c i y x", c=2, i=CSUB, y=10, x=10)

            rv, tv, hv, t2v = cview(res), cview(tbuf), cview(hbuf), cview(t2buf)
            scv = scC[:].rearrange("p (c i y x) -> p c i y x", c=2, i=CSUB, y=8, x=8)
            nblk = CSUB // 8

            def conv_c(wtile, src, dst_assign):
                for blk in range(nblk):
                    pss = []
                    for mt in range(2):
                        ps = pps.tile([128, 512], f32, tag="cps")
                        k = 0
                        for c in range(2):
                            for j in range(9):
                                ky, kx = OFFS[j]
                                nc.tensor.matmul(
                                    ps[:, :],
                                    wtile[:, (c * 9 + j) * 256 + mt * 128:(c * 9 + j) * 256 + mt * 128 + 128],
                                    src[:, c, blk * 8:blk * 8 + 8, ky:ky + 8, kx:kx + 8],
                                    start=(k == 0), stop=(k == 17))
                                k += 1
                        pss.append(ps)
                    for mt in range(2):
                        pv = pss[mt][:].rearrange("p (i y x) -> p i y x", i=8, y=8, x=8)
                        dst_assign(mt, blk, pv)

            wc2 = load_w("wC0c2")
            for c in range(2):
                nc.scalar.activation(t2v[:, c, :, 1:9, 1:9], rv[:, c, :, 1:9, 1:9], Relu,
                                     bias=bnB("C0bn2", c), scale=bnS("C0bn2", c))

            def asg0(mt, blk, pv):
                nc.vector.tensor_add(tv[:, mt, blk * 8:blk * 8 + 8, 1:9, 1:9], pv[:, :, :, :],
                                     scv[:, mt, blk * 8:blk * 8 + 8, :, :])
            conv_c(wc2, t2v, asg0)
            res, tbuf = tbuf, res
            rv, tv = tv, rv

            for bi in range(1, 4):
                wc1 = load_w(f"wC{bi}c1")
                wc2 = load_w(f"wC{bi}c2")
                for c in range(2):
                    nc.scalar.activation(tv[:, c, :, 1:9, 1:9], rv[:, c, :, 1:9, 1:9], Relu,
                                         bias=bnB(f"C{bi}bn1", c), scale=bnS(f"C{bi}bn1", c))

                def asg1(mt, blk, pv):
                    nc.scalar.copy(hv[:, mt, blk * 8:blk * 8 + 8, 1:9, 1:9], pv[:, :, :, :])
                conv_c(wc1, tv, asg1)
                for c in range(2):
                    nc.scalar.activation(t2v[:, c, :, 1:9, 1:9], hv[:, c, :, 1:9, 1:9], Relu,
                                         bias=bnB(f"C{bi}bn2", c), scale=bnS(f"C{bi}bn2", c))

                def asg2(mt, blk, pv):
                    nc.vector.tensor_add(tv[:, mt, blk * 8:blk * 8 + 8, 1:9, 1:9], pv[:, :, :, :],
                                         rv[:, mt, blk * 8:blk * 8 + 8, 1:9, 1:9])
                conv_c(wc2, t2v, asg2)
                res, tbuf = tbuf, res
                rv, tv = tv, rv

            for c in range(2):
                nc.scalar.activation(tv[:, c, :, 1:9, 1:9], rv[:, c, :, 1:9, 1:9], Relu,
                                     bias=bnB("bnf", c), scale=bnS("bnf", c))
            tg = tbuf[:].rearrange("p (g y x) -> p g y x", g=2 * CSUB, y=10, x=10)
            fv = fT[:].rearrange("p (c i) -> p c i", c=2, i=nimg)
            nc.vector.reduce_sum(fv[:, :, i0:i0 + CSUB], tg[:, :, 1:9, 1:9], axis=AX.XY)

        # ============ PHASE D: gate + experts ============
        NI = nimg
        nc.vector.tensor_copy(fTr[:], fT[:])

        emr = load_w("emr")
        fc1w = load_w("fc1w")
        fc2w = load_w("fc2w")

        psG = ppst.tile([128, 512], f32, tag="tps")
        for c in range(2):
            nc.tensor.matmul(psG[:, 0:NI], emr[:, 90 + c * 128:90 + (c + 1) * 128],
                             fTr[:, c * NI:(c + 1) * NI], start=(c == 0), stop=(c == 1))
        gh = psml.tile([128, 64], f32r, tag="gh")
        nc.scalar.activation(gh[:, 0:NI], psG[:, 0:NI], Relu, bias=emf[:, 48:49], scale=1.0)
        psL = ppst.tile([128, 512], f32, tag="tps")
        nc.tensor.matmul(psL[0:8, 0:NI], emr[:, 346:354], gh[:, 0:NI], start=True, stop=True)
        logT = psml.tile([128, 64], f32, tag="logT")
        nc.scalar.activation(logT[0:8, 0:NI], psL[0:8, 0:NI], Iden, bias=emf[0:8, 49:50], scale=1.0)

        psT = ppst.tile([128, 512], f32, tag="tps")
        nc.tensor.transpose(psT[0:NI, 0:8], logT[0:8, 0:NI], ident[0:8, 0:8])
        l_sb = psml.tile([64, 8], f32, tag="lsb")
        nc.vector.tensor_copy(l_sb[0:NI, :], psT[0:NI, 0:8])

        m1 = psml.tile([64, 1], f32, tag="m1")
        m2 = psml.tile([64, 1], f32, tag="m2")
        msk1 = psml.tile([64, 8], f32, tag="msk1")
        msk2 = psml.tile([64, 8], f32, tag="msk2")
        l2 = psml.tile([64, 8], f32, tag="l2")
        dd = psml.tile([64, 1], f32, tag="dd")
        e2 = psml.tile([64, 1], f32, tag="e2")
        den = psml.tile([64, 1], f32, tag="den")
        rr = psml.tile([64, 1], f32, tag="rr")
        w2g = psml.tile([64, 1], f32, tag="w2g")
        g1t = psml.tile([64, 8], f32, tag="g1t")
        gates = psml.tile([64, 8], f32, tag="gates")
        nc.vector.reduce_max(m1[0:NI, :], l_sb[0:NI, :], axis=AX.X)
        nc.vector.tensor_scalar(msk1[0:NI, :], l_sb[0:NI, :], m1[0:NI, :], None, Alu.is_equal)
        nc.vector.scalar_tensor_tensor(l2[0:NI, :], msk1[0:NI, :], -1e30, l_sb[0:NI, :], Alu.mult, Alu.add)
        nc.vector.reduce_max(m2[0:NI, :], l2[0:NI, :], axis=AX.X)
        nc.vector.tensor_scalar(msk2[0:NI, :], l2[0:NI, :], m2[0:NI, :], None, Alu.is_equal)
        nc.vector.tensor_sub(dd[0:NI, :], m2[0:NI, :], m1[0:NI, :])
        nc.scalar.activation(e2[0:NI, :], dd[0:NI, :], Exp)
        nc.vector.tensor_scalar_add(den[0:NI, :], e2[0:NI, :], 1.0)
        nc.vector.reciprocal(rr[0:NI, :], den[0:NI, :])
        nc.vector.tensor_mul(w2g[0:NI, :], e2[0:NI, :], rr[0:NI, :])
        nc.vector.tensor_scalar(g1t[0:NI, :], msk1[0:NI, :], rr[0:NI, :], None, Alu.mult)
        nc.vector.scalar_tensor_tensor(gates[0:NI, :], msk2[0:NI, :], w2g[0:NI, :], g1t[0:NI, :],
                                       Alu.mult, Alu.add)

        psT2 = ppst.tile([128, 512], f32, tag="tps")
        nc.tensor.transpose(psT2[0:8, 0:NI], gates[0:NI, 0:8], ident[0:NI, 0:NI])
        gTr = psml.tile([128, 64], f32r, tag="gTr")
        nc.vector.tensor_copy(gTr[0:8, 0:NI], psT2[0:8, 0:NI])

        h1 = psml.tile([128, 128], f32r, tag="h1")
        h2 = psml.tile([128, 64], f32r, tag="h2")
        accA = psml.tile([64, 10], f32, tag="accA")
        accB = psml.tile([64, 10], f32, tag="accB")

        psb = ppse.tile([128, 512], f32, tag="eps")
        nc.tensor.matmul(psb[0:NI, 0:10], gTr[0:8, 0:NI], emr[0:8, 80:90], start=True, stop=True)
        nc.vector.tensor_copy(accA[0:NI, :], psb[0:NI, 0:10])
        src, dst = accA, accB
        for e in range(8):
            for mt in range(2):
                ps1 = ppse.tile([128, 512], f32, tag="eps")
                for c in range(2):
                    nc.tensor.matmul(ps1[:, 0:NI],
                                     fc1w[:, ((e * 2 + c) * 2 + mt) * 128:((e * 2 + c) * 2 + mt + 1) * 128],
                                     fTr[:, c * NI:(c + 1) * NI], start=(c == 0), stop=(c == 1))
                nc.scalar.activation(h1[:, mt * 64:mt * 64 + NI], ps1[:, 0:NI], Relu,
                                     bias=emf[:, 16 + e * 2 + mt:17 + e * 2 + mt],
                                     scale=emf[:, e * 2 + mt:e * 2 + mt + 1])
            ps2 = ppse.tile([128, 512], f32, tag="eps")
            for hc in range(2):
                nc.tensor.matmul(ps2[:, 0:NI], fc2w[:, (e * 2 + hc) * 128:(e * 2 + hc + 1) * 128],
                                 h1[:, hc * 64:hc * 64 + NI], start=(hc == 0), stop=(hc == 1))
            nc.scalar.activation(h2[:, 0:NI], ps2[:, 0:NI], Relu,
                                 bias=emf[:, 40 + e:41 + e], scale=emf[:, 32 + e:33 + e])
            ps3 = ppse.tile([128, 512], f32, tag="eps")
            nc.tensor.matmul(ps3[0:NI, 0:10], h2[:, 0:NI], emr[:, e * 10:(e + 1) * 10],
                             start=True, stop=True)
            nc.vector.scalar_tensor_tensor(dst[0:NI, :], ps3[0:NI, 0:10], gates[0:NI, e:e + 1],
                                           src[0:NI, :], Alu.mult, Alu.add)
            src, dst = dst, src

        nc.sync.dma_start(d["out"][:, :], src[0:NI, :])

    nc.compile()
    return nc


# ---------------------------------------------------------------------------
# Public entry point
# ---------------------------------------------------------------------------

_CACHE = {}


def kernel(x, params):
    from concourse.bass_utils import run_bass_kernel_spmd

    x = np.asarray(x, dtype=np.float32)
    B = x.shape[0]
    per = B // N_CORES
    assert per == 64, f"expected 64 imgs/core, got {per}"

    if "nc" not in _CACHE:
        shared, colmap = prep_shared(params)
        nc = build_nc(colmap, nimg=64)
        _CACHE["nc"] = (nc, shared)
    nc, shared = _CACHE["nc"]

    in_maps = []
    for c in range(N_CORES):
        m = dict(shared)
        m["xcol"] = prep_xcol(x[c * per:(c + 1) * per])
        in_maps.append(m)

    res = run_bass_kernel_spmd(nc, in_maps, core_ids=list(range(N_CORES)))
    out = np.concatenate([res.results[c]["out"] for c in range(N_CORES)], axis=0)
    return out.astype(np.float32)
